# revision 1
# baseline (speedup 1.0000x reference)
"""Distributed Trainium2 (Bass) kernel for nn_ABDMBR (multi-behavior LightGCN + BPR).

8 NeuronCores SPMD. v2 design:
  - Node blocks (128 nodes) assigned round-robin to cores (balanced edges).
  - Within-shard layout p-major: phys_row(n) = owner*SH + (n%128)*NBLK + (n>>7)//8.
  - Host computes degrees, rs=rsqrt(deg), item weights wn; tables pre-scaled
    where possible so most one-hots are plain (batched is_equal on DVE).
  - Per pass: 4 sweeps x 4 pieces; edges sorted (sweep, piece, block); bins
    equalized across cores with dummy edges (idx=0, dl=200); chunks of 128
    cross block boundaries (boundary chunks get 2 matmul incidences).
  - PSUM: [128, sweep_blocks, 64] accumulators; drained via ACT activation
    (Copy, scale=rs[dst]) to an f32 stage; finalize batched on DVE per sweep.
  - AllGathers: T1, AE, TB1[b] full tables [SH,128] bf16 rows.
  - Tail: per-core staging of owned rows from local spilled tables, one small
    AllGather of staging, batch-sharded tail compute, AllReduce of the loss.
  - Embedding-norm regularization term computed on host, added post-device.
"""

import os
import numpy as np
import ml_dtypes

bf16 = ml_dtypes.bfloat16


class Cfg:
    def __init__(self, NU, NI, E, BATCH):
        self.NU, self.NI, self.E, self.BATCH = NU, NI, E, BATCH
        self.D, self.B, self.LAYERS = 64, 3, 2
        self.NN = NU + NI
        self.NCORES = 8
        nblk_tot = -(-self.NN // 128)          # logical blocks
        nblk_tot = -(-nblk_tot // 8) * 8       # round to multiple of 8
        self.NBLK = nblk_tot // 8              # blocks per core
        self.SH = self.NBLK * 128
        self.NPAD = self.SH * 8
        # sweeps of ~25 blocks
        self.NSW = 5 if self.NBLK >= 10 else 1
        base = self.NBLK // self.NSW
        rem = self.NBLK % self.NSW
        self.sweep_blocks = [base + (1 if i < rem else 0) for i in range(self.NSW)]
        self.sweep_start = np.concatenate([[0], np.cumsum(self.sweep_blocks)[:-1]]).astype(int)
        self.PIECE = 32768
        self.NPIECE = -(-self.NPAD // self.PIECE)
        self.GCALL = 40                        # chunks per gather call
        self.ROWC = 128                        # table row cols (bf16) = 256B
        self.NT = self.BATCH // 128            # batch tiles per behavior
        ns = self.NT * self.B                  # total slices
        assert ns % self.NCORES == 0
        self.SLICES_PER_CORE = ns // self.NCORES

    # node -> (owner core, local block j, partition p)
    def node_loc(self, n):
        lb = n >> 7
        return lb % 8, lb // 8, n & 127

    def phys_row(self, n):
        lb = n >> 7
        return (lb % 8) * self.SH + (n & 127) * self.NBLK + (lb // 8)


FULL = Cfg(60001, 40001, 600000, 2048)


# ================================================================ host prep

def _wrap16(v):
    """idx list -> [16, n/16] int16 layout (slot q = arr[q%16, q//16])."""
    v = np.asarray(v)
    n = len(v)
    assert n % 16 == 0
    return np.ascontiguousarray(v.reshape(-1, 16).T.astype(np.int16))


class PassLayout:
    """Edge schedule for one graph pass, shared across cores (SPMD).

    Edges binned by (sweep, piece, block); bins equalized across cores with
    dummies; runs (sweep,piece) padded to x128; chunks cross block bounds.
    """

    def __init__(self, cfg, src_phys, dst_phys, rs_src=None):
        C = cfg
        owner = (dst_phys // C.SH).astype(np.int64)
        rem = dst_phys % C.SH
        p_dst = rem // C.NBLK          # partition = n%128
        j_dst = rem % C.NBLK           # local block
        sweep = np.searchsorted(C.sweep_start, j_dst, side="right") - 1
        piece = src_phys // C.PIECE
        # bin id: (sweep, piece, block)
        nbin_per_sp = C.NBLK  # indexed by j, but only sweep's j range used
        binid = (sweep * C.NPIECE + piece) * C.NBLK + j_dst
        NBIN = C.NSW * C.NPIECE * C.NBLK
        key = owner * NBIN + binid
        order = np.lexsort((src_phys, key))
        s_src, s_j, s_p, s_key = (src_phys[order], j_dst[order],
                                  p_dst[order], key[order])
        s_rs = rs_src[order] if rs_src is not None else None
        counts = np.bincount(s_key, minlength=C.NCORES * NBIN).reshape(C.NCORES, NBIN)
        bin_eq = counts.max(axis=0).copy()     # equalized per-bin count
        # every (sweep, j-in-sweep) block needs >=1 slot so its PSUM chain
        # and drain exist: bump piece-0 bin if the whole block is empty.
        be = bin_eq.reshape(C.NSW, C.NPIECE, C.NBLK)
        for s in range(C.NSW):
            j0, j1 = C.sweep_start[s], C.sweep_start[s] + C.sweep_blocks[s]
            empty = be[s].sum(axis=0)[j0:j1] == 0
            be[s, 0, j0:j1][empty] = 1
            be[s, :, :j0] = 0
            be[s, :, j1:] = 0
        bin_eq = be.reshape(-1)
        # pad each (sweep,piece) run to x128 (into last block of the sweep)
        run_of_bin = np.repeat(np.arange(C.NSW * C.NPIECE), C.NBLK)
        NRUN = C.NSW * C.NPIECE
        run_tot = np.zeros(NRUN, np.int64)
        np.add.at(run_tot, run_of_bin, bin_eq)
        run_pad = (-run_tot) % 128
        bin_eq2 = bin_eq.copy()
        for r in range(NRUN):
            s = r // C.NPIECE
            j_last = C.sweep_start[s] + C.sweep_blocks[s] - 1
            bin_eq2[r * C.NBLK + j_last] += run_pad[r]
        self.bin_eq = bin_eq2
        bin_start = np.concatenate([[0], np.cumsum(bin_eq2)[:-1]])
        run_len = run_tot + run_pad
        run_start = np.concatenate([[0], np.cumsum(run_len)[:-1]])
        self.slots = int(run_len.sum())
        assert self.slots % 128 == 0
        self.nch = self.slots // 128

        # ---- per-core idx arrays + slot of each real edge
        core_rank = np.arange(len(s_src)) - np.repeat(
            np.concatenate([[0], np.cumsum(counts.reshape(-1))[:-1]]),
            counts.reshape(-1))
        slot = bin_start[s_key % NBIN] + core_rank
        self.idx = []
        base = (s_src % C.PIECE).astype(np.int16)
        for c in range(C.NCORES):
            m = (s_key // NBIN) == c
            iv = np.zeros(self.slots, np.int16)
            iv[slot[m]] = base[m]
            self.idx.append(_wrap16(iv))

        # ---- incidences: per chunk, blocks intersecting
        # bin (sweep,piece,j) occupies slots [bin_start, bin_start+bin_eq2)
        incs = []   # (chunk, j, first, last) -- first/last resolved later
        # iterate runs in order
        chunk_of_slot0 = None
        inc_for_binpart = {}  # (binidx, chunk) -> inc index
        for r in range(NRUN):
            st, ln = run_start[r], run_len[r]
            if ln == 0:
                continue
            c0, c1 = st // 128, (st + ln) // 128
            sweep, piece = r // C.NPIECE, r % C.NPIECE
            for ch in range(c0, c1):
                lo, hi = ch * 128, ch * 128 + 128
                # blocks whose bin ranges intersect [lo,hi)
                jlo = np.searchsorted(bin_start[r * C.NBLK:(r + 1) * C.NBLK] +
                                      bin_eq2[r * C.NBLK:(r + 1) * C.NBLK], lo,
                                      side="right")
                jhi = np.searchsorted(bin_start[r * C.NBLK:(r + 1) * C.NBLK], hi,
                                      side="left")
                for j in range(jlo, jhi):
                    if bin_eq2[r * C.NBLK + j] == 0:
                        continue
                    incs.append([ch, j, sweep, piece])
                    inc_for_binpart[(r * C.NBLK + j, ch)] = len(incs) - 1
        self.incs = incs
        self.n_inc = len(incs)

        # first/last per (sweep, j): chain over pieces
        firsts = {}
        lasts = {}
        for i, (ch, j, sw, pc) in enumerate(incs):
            k = (sw, j)
            if k not in firsts:
                firsts[k] = i
            lasts[k] = i
        self.inc_first = np.zeros(self.n_inc, bool)
        self.inc_last = np.zeros(self.n_inc, bool)
        for k, i in firsts.items():
            self.inc_first[i] = True
        for k, i in lasts.items():
            self.inc_last[i] = True

        # ---- per-core dl (and rs) arrays indexed by incidence
        n_inc_pad = -(-self.n_inc // 8) * 8
        self.n_inc_pad = n_inc_pad
        self.dl = []
        self.rse = [] if rs_src is not None else None
        # edge -> incidence: binidx = key%NBIN, chunk = slot//128
        e_bin = (s_key % NBIN)
        e_ch = slot // 128
        e_inc = np.array([inc_for_binpart[(b, c)] for b, c in
                          zip(e_bin, e_ch)], np.int64)
        e_p = slot % 128
        for c in range(C.NCORES):
            m = (s_key // NBIN) == c
            dlv = np.full((128, n_inc_pad), 200.0, np.float32)
            dlv[e_p[m], e_inc[m]] = s_p[m]
            self.dl.append(np.ascontiguousarray(
                dlv if rs_src is not None else dlv.astype(bf16)))
            if rs_src is not None:
                rv = np.zeros((128, n_inc_pad), np.float32)
                rv[e_p[m], e_inc[m]] = s_rs[m]
                self.rse.append(np.ascontiguousarray(rv))

        # ---- gather calls: per run, groups of <=GCALL chunks
        self.calls = []   # (piece, chunk0, nch)
        for r in range(NRUN):
            st, ln = run_start[r], run_len[r]
            if ln == 0:
                continue
            piece = r % C.NPIECE
            c0, c1 = st // 128, (st + ln) // 128
            ch = c0
            while ch < c1:
                n = min(C.GCALL, c1 - ch)
                self.calls.append((int(piece), int(ch), int(n)))
                ch += n
        # map chunk -> (call_index, offset) for matmul rhs lookup
        self.chunk_call = {}
        for ci, (pc, c0, n) in enumerate(self.calls):
            for k in range(n):
                self.chunk_call[c0 + k] = (ci, k)


class TailPlan:
    """Staging + sharded tail compute plan."""

    def __init__(self, cfg, batch_data, loc_of_node):
        C = cfg
        bd = np.asarray(batch_data, np.int64)
        B, NT = C.B, C.NT
        # ---- user requests: (b, slot, bb) -> node bd[slot,b,0]
        users = bd[:, :, 0]                      # [BATCH, B] user node id
        # items offset into global node space
        items = bd[:, :, 1:3] + C.NU             # [BATCH, B, 2]

        uloc = loc_of_node(users)                # (owner, locoff) each [BATCH,B]
        iloc = loc_of_node(items)

        # --- group user reqs by owner; per (owner, bb) groups padded equally
        uo, ul = uloc                             # owner, local 128B-unit offset
        # per-core per-bb request lists (same nodes for all bb)
        cnt = np.bincount(uo.reshape(-1), minlength=8)
        grp = int(-(-(cnt.max()) // 128) * 128)  # per-bb group size
        self.u_grp = grp
        self.u_cols = 3 * (grp // 128)           # staged cols per partition
        umax = 3 * grp
        self.umax = umax
        # per core: for bb in 0..2: idx list (grp) of local PAIR idx + parity
        self.u_idx = []      # per core: [16, 3*grp/16]
        self.u_par = []      # per core: [128, 3*(grp//128)] bf16 parity
        # global staged position of request (b, slot, bb):
        self.u_pos = np.zeros((C.BATCH, B, 3), np.int64)
        order_by_core = [np.nonzero(uo.reshape(-1) == c)[0] for c in range(8)]
        for c in range(8):
            sel = order_by_core[c]               # flat (slot*B+b) indices
            locs = ul.reshape(-1)[sel]
            npad = grp - len(sel)
            locs_p = np.concatenate([locs, np.zeros(npad, np.int64)])
            idx_bb = []
            par_bb = []
            for bb in range(3):
                idx_bb.append((locs_p >> 1).astype(np.int16))
                par = (locs_p & 1).astype(np.float32)
                par_bb.append(par.reshape(-1, 128).T)   # [128, grp//128]
            self.u_idx.append(_wrap16(np.concatenate(idx_bb)))
            self.u_par.append(np.ascontiguousarray(
                np.concatenate(par_bb, axis=1).astype(bf16)))
            # staged pos: request q of group bb on core c:
            # gather slot q -> (p=q%128, t=q//128); staged col = bb*(grp//128)+t
            # staged row (p-major within core) = p*u_cols + col
            # global = c*umax + staged row
            q = np.arange(len(sel))
            p, t = q % 128, q // 128
            for bb in range(3):
                col = bb * (grp // 128) + t
                self.u_pos.reshape(-1, 3)[sel, bb] = c * umax + p * self.u_cols + col

        # --- item requests: (b, slot, k)
        io_, il = iloc
        cnt = np.bincount(io_.reshape(-1), minlength=8)
        grp = int(-(-(cnt.max()) // 128) * 128)
        self.i_grp = grp
        self.i_cols = grp // 128
        self.imax = grp
        self.i_idx = []
        self.i_par = []
        flat_pos = np.zeros(C.BATCH * B * 2, np.int64)
        for c in range(8):
            sel = np.nonzero(io_.reshape(-1) == c)[0]
            locs = il.reshape(-1)[sel]
            npad = grp - len(sel)
            locs_p = np.concatenate([locs, np.zeros(npad, np.int64)])
            self.i_idx.append(_wrap16((locs_p >> 1).astype(np.int16)))
            self.i_par.append(np.ascontiguousarray(
                ((locs_p & 1).astype(np.float32)).reshape(-1, 128).T.astype(bf16)))
            q = np.arange(len(sel))
            p, t = q % 128, q // 128
            flat_pos[sel] = c * grp + p * self.i_cols + t
        self.i_pos = flat_pos.reshape(C.BATCH, B, 2)

        # --- per-slice compute gather lists (per core)
        # slice s (global 0..NT*B-1): t = s // B, b = s % B ... assign so each
        # core gets SLICES_PER_CORE slices: core c takes s in [c*spc,(c+1)*spc)
        spc = C.SLICES_PER_CORE
        self.spc = spc
        self.slice_u_idx = []   # per core: [16, spc*3*128/16]
        self.slice_u_par = []   # per core: [128, spc*3] bf16
        self.slice_i_idx = []
        self.slice_i_par = []
        for c in range(8):
            uidx_all, upar_all, iidx_all, ipar_all = [], [], [], []
            for s in range(c * spc, (c + 1) * spc):
                t, b = s // B, s % B
                slots = t * 128 + np.arange(128)
                for bb in range(3):
                    g = self.u_pos[slots, b, bb]
                    uidx_all.append((g >> 1).astype(np.int16))
                    upar_all.append((g & 1).astype(np.float32))
                for k in range(2):
                    g = self.i_pos[slots, b, k]
                    iidx_all.append((g >> 1).astype(np.int16))
                    ipar_all.append((g & 1).astype(np.float32))
            self.slice_u_idx.append(_wrap16(np.concatenate(uidx_all)))
            self.slice_u_par.append(np.ascontiguousarray(
                np.stack(upar_all, axis=1).astype(bf16)))  # [128, spc*3]
            self.slice_i_idx.append(_wrap16(np.concatenate(iidx_all)))
            self.slice_i_par.append(np.ascontiguousarray(
                np.stack(ipar_all, axis=1).astype(bf16)))
        # per-slice attention row weights (w0,w1,w2): data-driven so the SPMD
        # program does not branch on the slice's behavior.
        self.slw = []
        for c in range(8):
            w = np.zeros((128, spc * 3), np.float32)
            for sr in range(spc):
                b = (c * spc + sr) % B
                if b < 2:
                    w[:, sr * 3 + b] = 1.0
                else:
                    w[:, sr * 3:sr * 3 + 3] = 1.0
            self.slw.append(np.ascontiguousarray(w))


def _host_prep(cfg, user_emb, item_emb, W, edge_users, edge_items, batch_data):
    C = cfg
    meta = {"cfg": C}
    ue = np.asarray(user_emb, np.float32)
    ie = np.asarray(item_emb, np.float32)
    eu = np.asarray(edge_users, np.int64)
    ei = np.asarray(edge_items, np.int64)
    Wv = np.asarray(W, np.float32)

    emb0 = np.concatenate([ue, ie], axis=0)          # [NN, D]

    # ---- degrees / rs per graph (host)
    def deg_rs(src, dst):
        deg = np.bincount(src, minlength=C.NN).astype(np.float32)
        # symmetric graph: src list == union of both directions
        rs = 1.0 / np.sqrt(np.maximum(deg, 1.0))
        return deg, rs

    srcs, dsts = [], []
    for b in range(C.B):
        srcs.append(np.concatenate([eu[b], ei[b] + C.NU]))
        dsts.append(np.concatenate([ei[b] + C.NU, eu[b]]))
    g_src = np.concatenate(srcs)
    g_dst = np.concatenate(dsts)
    _, rs_g = deg_rs(g_src, g_dst)
    rs_b = []
    for b in range(C.B):
        _, r = deg_rs(srcs[b], dsts[b])
        rs_b.append(r)

    # ---- item behaviour weights wn [NN] per b (nonzero on item rows)
    ibd = np.stack([np.bincount(ei[b], minlength=C.NI).astype(np.float32)
                    for b in range(C.B)], axis=1)     # [NI, B]
    wt = ibd * Wv
    wn_items = wt / (wt.sum(axis=1, keepdims=True) + 1e-8)   # [NI, B]
    wn_full = np.zeros((C.NN, C.B), np.float32)
    wn_full[C.NU:] = wn_items

    # ---- node -> phys mapping helpers
    def phys(n):
        lb = n >> 7
        return (lb % 8) * C.SH + (n & 127) * C.NBLK + (lb // 8)

    # per-core [128, NBLK] arrays: entry (p, j) = node of (core, j, p)
    def per_core_blk(vals, fill=0.0):
        # vals [NN] -> list of [128, NBLK] f32
        out = []
        n = np.arange(C.NN)
        lb = n >> 7
        for c in range(C.NCORES):
            a = np.full((128, C.NBLK), fill, np.float32)
            m = (lb % 8) == c
            a[n[m] & 127, lb[m] // 8] = vals[m]
            out.append(np.ascontiguousarray(a))
        return out

    meta["rs_g_blk"] = per_core_blk(rs_g)
    meta["rs_b_blk"] = [per_core_blk(rs_b[b]) for b in range(C.B)]
    meta["wn_blk"] = [per_core_blk(wn_full[:, b]) for b in range(C.B)]

    # ---- t0g table (pre-scaled by rs_g), [NPAD, ROWC] bf16
    T0g = np.zeros((C.NPAD, C.ROWC), bf16)
    n = np.arange(C.NN)
    T0g[phys(n), :C.D] = (emb0 * rs_g[:, None]).astype(bf16)
    meta["T0g"] = T0g

    # ---- t0acc per core: [128, NBLK*64] f32 raw emb
    meta["t0acc"] = []
    lb = n >> 7
    for c in range(C.NCORES):
        a = np.zeros((128, C.NBLK, C.D), np.float32)
        m = (lb % 8) == c
        a[n[m] & 127, lb[m] // 8] = emb0[m]
        meta["t0acc"].append(np.ascontiguousarray(a.reshape(128, -1)))

    # ---- pass layouts
    gs, gd = phys(g_src), phys(g_dst)
    meta["lay_g"] = PassLayout(C, gs, gd)
    meta["lay_b1"] = []   # behavior L1 (scaled one-hots)
    meta["lay_b2"] = []   # behavior L2 (plain)
    for b in range(C.B):
        sp, dp = phys(srcs[b]), phys(dsts[b])
        meta["lay_b1"].append(PassLayout(C, sp, dp, rs_src=rs_b[b][srcs[b]]))
        meta["lay_b2"].append(PassLayout(C, sp, dp))

    # ---- tail plan
    def loc_of_node(nodes):
        ph = phys(nodes)
        owner = ph // C.SH
        return owner, ph % C.SH      # local 128B-unit offset (p-major)

    meta["tail"] = TailPlan(C, batch_data, loc_of_node)

    # ---- host regularization term
    meta["reg"] = float(1e-3 * (np.linalg.norm(ue) + np.linalg.norm(ie)) / C.NI)
    return meta


# ================================================================ device build

def _build_graph(meta):
    C = meta["cfg"]
    import concourse.bacc as bacc
    import concourse.tile as tile
    from concourse import mybir

    f32 = mybir.dt.float32
    bt = mybir.dt.bfloat16
    i16 = mybir.dt.int16
    AF = mybir.ActivationFunctionType
    OP = mybir.AluOpType
    RG = [list(range(C.NCORES))]
    D, B, NBLK, SH = C.D, C.B, C.NBLK, C.SH
    TP = meta["tail"]
    PH = int(os.environ.get("GNN_PH", "9"))

    nc = bacc.Bacc(None, num_swdge_queues=4)

    # ---------------- params
    t0g = nc.declare_dram_parameter("t0g", [C.NPAD, C.ROWC], bt, isOutput=False)
    t0acc = nc.declare_dram_parameter("t0acc", [128, NBLK * D], f32, isOutput=False)
    lay_all = [("g1", meta["lay_g"], False), ("g2", meta["lay_g"], False)]
    for b in range(B):
        lay_all.append((f"b{b}l1", meta["lay_b1"][b], True))
    for b in range(B):
        lay_all.append((f"b{b}l2", meta["lay_b2"][b], False))
    p_idx, p_dl, p_rs = {}, {}, {}
    for name, lay, scaled in lay_all:
        if name == "g2":   # shares layout arrays with g1
            continue
        p_idx[name] = nc.declare_dram_parameter(
            f"idx_{name}", [16, lay.slots // 16], i16, isOutput=False)
        p_dl[name] = nc.declare_dram_parameter(
            f"dl_{name}", [128, lay.n_inc_pad], f32 if scaled else bt,
            isOutput=False)
        if scaled:
            p_rs[name] = nc.declare_dram_parameter(
                f"rs_{name}", [128, lay.n_inc_pad], f32, isOutput=False)
    rs_g_p = nc.declare_dram_parameter("rs_g_blk", [128, NBLK], f32, isOutput=False)
    rs_b_p = [nc.declare_dram_parameter(f"rs_b{b}_blk", [128, NBLK], f32, isOutput=False)
              for b in range(B)]
    wn_p = [nc.declare_dram_parameter(f"wn{b}_blk", [128, NBLK], f32, isOutput=False)
            for b in range(B)]
    # tail params
    tu_idx = nc.declare_dram_parameter("tu_idx", [16, 3 * TP.u_grp // 16], i16, isOutput=False)
    tu_par = nc.declare_dram_parameter("tu_par", [128, TP.u_cols], bt, isOutput=False)
    ti_idx = nc.declare_dram_parameter("ti_idx", [16, TP.i_grp // 16], i16, isOutput=False)
    ti_par = nc.declare_dram_parameter("ti_par", [128, TP.i_cols], bt, isOutput=False)
    su_idx = nc.declare_dram_parameter("su_idx", [16, TP.spc * 3 * 128 // 16], i16, isOutput=False)
    su_par = nc.declare_dram_parameter("su_par", [128, TP.spc * 3], bt, isOutput=False)
    si_idx = nc.declare_dram_parameter("si_idx", [16, TP.spc * 2 * 128 // 16], i16, isOutput=False)
    si_par = nc.declare_dram_parameter("si_par", [128, TP.spc * 2], bt, isOutput=False)
    slw_p = nc.declare_dram_parameter("slw", [128, TP.spc * 3], f32, isOutput=False)
    out_p = nc.declare_dram_parameter("out", [1, 1], f32, isOutput=True)

    # ---------------- internal dram
    ag_in = nc.dram_tensor("ag_in", [SH, C.ROWC], bt)
    T1 = nc.dram_tensor("T1", [C.NPAD, C.ROWC], bt, addr_space="Shared")
    AE = nc.dram_tensor("AE", [C.NPAD, C.ROWC], bt, addr_space="Shared")
    TB1 = [nc.dram_tensor(f"TB1_{b}", [C.NPAD, C.ROWC], bt, addr_space="Shared")
           for b in range(B)]
    FBloc = [nc.dram_tensor(f"FBloc{b}", [128, NBLK * D], bt) for b in range(B)]
    TBloc = [nc.dram_tensor(f"TBloc{b}", [128, NBLK * D], bt) for b in range(B)]
    IFloc = nc.dram_tensor("IFloc", [128, NBLK * D], bt)
    stage_u_in = nc.dram_tensor("stage_u_in", [TP.umax, D], bt)
    stage_u_full = nc.dram_tensor("stage_u_full", [8 * TP.umax, D], bt, addr_space="Shared")
    stage_i_in = nc.dram_tensor("stage_i_in", [TP.imax, D], bt)
    stage_i_full = nc.dram_tensor("stage_i_full", [8 * TP.imax, D], bt, addr_space="Shared")
    loss_in = nc.dram_tensor("loss_in", [1, 1], f32)
    loss_out = nc.dram_tensor("loss_out", [1, 1], f32, addr_space="Shared")

    def next_q():
        # all SWDGE ops on one queue: the Tile scheduler assigns DMASW sem
        # lanes in *scheduled* order, which a rotating queue_num cannot track.
        return 0

    with tile.TileContext(nc) as tc:
        with (
            tc.tile_pool(name="mp", bufs=1) as mp,
            tc.tile_pool(name="wp", bufs=2) as wp,
            tc.tile_pool(name="gp", bufs=3) as gp,
            tc.tile_pool(name="ohp", bufs=3) as ohp,
            tc.tile_pool(name="stp", bufs=2) as stp,
            tc.tile_pool(name="pp", bufs=2, space="PSUM") as pp,
        ):
            # constants
            iota_np = np.tile(np.arange(128, dtype=np.float32), (128, 8))
            iotaK = mp.tile([128, 8 * 128], bt)
            nc.sync.dma_start(out=iotaK[:], in_=nc.inline_tensor(
                iota_np.astype(bf16), name="iotaK")[:])
            ones_f = mp.tile([128, 1], f32)
            nc.vector.memset(ones_f[:], 1.0)

            # persistent state
            acc = mp.tile([128, NBLK, D], f32)
            if_acc = mp.tile([128, NBLK, D], bt)
            rs_g_t = mp.tile([128, NBLK], f32, name="rsg", tag="rsg")
            nc.sync.dma_start(out=rs_g_t[:], in_=rs_g_p[:])
            rs_b_t, wn_t = [], []
            for b in range(B):
                t = mp.tile([128, NBLK], f32, name=f"rsb{b}", tag=f"rsb{b}")
                nc.sync.dma_start(out=t[:], in_=rs_b_p[b][:])
                rs_b_t.append(t)
                t = mp.tile([128, NBLK], f32, name=f"wn{b}", tag=f"wn{b}")
                nc.sync.dma_start(out=t[:], in_=wn_p[b][:])
                wn_t.append(t)
            nc.sync.dma_start(
                out=acc[:].rearrange("p b d -> p (b d)"), in_=t0acc[:])
            nc.vector.memset(if_acc[:], 0.0)

            def bcast(t, j0, nb):
                return t[:, j0:j0 + nb, None].to_broadcast([128, nb, D])

            # ---------------- generic pass
            def emit_pass(name, lay, scaled, table, rs_t, fin):
                dl_t = wp.tile([128, lay.n_inc_pad], f32 if scaled else bt,
                               name="dl_t", tag="dl")
                nc.sync.dma_start(out=dl_t[:, :lay.n_inc_pad],
                                  in_=p_dl["g1" if name == "g2" else name][:])
                if scaled:
                    rse_t = wp.tile([128, lay.n_inc_pad], f32, name="rse_t",
                                    tag="rse")
                    nc.sync.dma_start(out=rse_t[:], in_=p_rs[name][:])
                idxp = p_idx["g1" if name == "g2" else name]
                # incidences grouped per call
                call_incs = [[] for _ in lay.calls]
                for i, (ch, j, sw, pc) in enumerate(lay.incs):
                    ci, off = lay.chunk_call[ch]
                    call_incs[ci].append((i, off, j, sw))
                drains_left = {sw: C.sweep_blocks[sw] for sw in range(C.NSW)}
                psum = {}
                stage = {}
                for ci, (piece, c0, nch) in enumerate(lay.calls):
                    idxt = wp.tile([128, C.GCALL * 8], i16, tag="idxt")
                    nc.vector.memset(idxt[:], 0)
                    nc.sync.dma_start(out=idxt[0:16, :nch * 8],
                                      in_=idxp[:, c0 * 8:(c0 + nch) * 8])
                    gt = gp.tile([128, C.GCALL, C.ROWC], bt, tag="gt")
                    pb = piece * C.PIECE
                    pe = min(C.NPAD, pb + C.PIECE)
                    nc.gpsimd.dma_gather(
                        out_ap=gt[:, :nch, :],
                        in_ap=table[pb:pe, :],
                        idxs_ap=idxt[:, :nch * 8],
                        num_idxs=nch * 128,
                        num_idxs_reg=nch * 128,
                        elem_size=C.ROWC,
                        single_packet=False,
                        queue_num=next_q(),
                    )
                    incs = call_incs[ci]
                    for g0 in range(0, len(incs), 8):
                        grp = incs[g0:g0 + 8]
                        k = len(grp)
                        i0 = grp[0][0]
                        oh = ohp.tile([128, 8 * 128], bt, tag="oh")
                        contig = all(grp[z][0] == i0 + z for z in range(k))
                        if not scaled and contig:
                            nc.vector.tensor_tensor(
                                out=oh[:, :k * 128].rearrange("p (k c) -> p k c", k=k),
                                in0=dl_t[:, i0:i0 + k, None].to_broadcast([128, k, 128]),
                                in1=iotaK[:, :k * 128].rearrange("p (k c) -> p k c", k=k),
                                op=OP.is_equal,
                            )
                        else:
                            for z, (i, off, j, sw) in enumerate(grp):
                                if scaled:
                                    nc.vector.tensor_scalar(
                                        out=oh[:, z * 128:(z + 1) * 128],
                                        in0=iotaK[:, :128],
                                        scalar1=dl_t[:, i:i + 1],
                                        scalar2=rse_t[:, i:i + 1],
                                        op0=OP.is_equal, op1=OP.mult)
                                else:
                                    nc.vector.tensor_scalar(
                                        out=oh[:, z * 128:(z + 1) * 128],
                                        in0=iotaK[:, :128],
                                        scalar1=dl_t[:, i:i + 1],
                                        scalar2=None, op0=OP.is_equal)
                        for z, (i, off, j, sw) in enumerate(grp):
                            if sw not in psum:
                                nb = C.sweep_blocks[sw]
                                psum[sw] = pp.tile([128, nb, D], f32, name=f"ps{sw}", tag="ps")
                                stage[sw] = stp.tile([128, nb, D], bt, name=f"stage{sw}", tag="stage")
                            jr = j - int(C.sweep_start[sw])
                            nc.tensor.matmul(
                                out=psum[sw][:, jr, :],
                                lhsT=oh[:, z * 128:(z + 1) * 128],
                                rhs=gt[:, off, 0:D],
                                start=bool(lay.inc_first[i]),
                                stop=bool(lay.inc_last[i]),
                            )
                            if lay.inc_last[i]:
                                nc.scalar.activation(
                                    out=stage[sw][:, jr, :],
                                    in_=psum[sw][:, jr, :],
                                    func=AF.Copy,
                                    scale=rs_t[:, j:j + 1])
                                drains_left[sw] -= 1
                                if drains_left[sw] == 0:
                                    # finalize this sweep NOW so pool buffers
                                    # (bufs=2) can be reused 2 sweeps later
                                    fin(sw, int(C.sweep_start[sw]),
                                        C.sweep_blocks[sw], stage[sw])

            def ag_view(j0, nb):
                return ag_in[:, 0:D].rearrange(
                    "(p j) c -> p j c", p=128)[:, j0:j0 + nb, :]

            def do_ag(dst):
                nc.gpsimd.collective_compute(
                    "AllGather", OP.bypass, replica_groups=RG,
                    ins=[ag_in[:]], outs=[dst[:]])

            # ---------------- finalizers
            def fin_g1(sw, j0, nb, stage):
                nc.vector.tensor_tensor(
                    out=acc[:, j0:j0 + nb, :], in0=acc[:, j0:j0 + nb, :],
                    in1=stage[:], op=OP.add)
                sb = wp.tile([128, nb, D], bt, name="sb_g1", tag="sb16")
                nc.vector.tensor_tensor(
                    out=sb[:], in0=stage[:], in1=bcast(rs_g_t, j0, nb), op=OP.mult)
                nc.sync.dma_start(out=ag_view(j0, nb), in_=sb[:])

            def fin_g2(sw, j0, nb, stage):
                st32 = wp.tile([128, nb, D], f32, name="st32", tag="st32")
                nc.vector.tensor_tensor(
                    out=st32[:], in0=stage[:], in1=acc[:, j0:j0 + nb, :], op=OP.add)
                nc.vector.tensor_scalar_mul(
                    acc[:, j0:j0 + nb, :], st32[:], 1.0 / 3.0)
                sb = wp.tile([128, nb, D], bt, name="sb_g2", tag="sb16")
                nc.vector.tensor_copy(out=sb[:], in_=acc[:, j0:j0 + nb, :])
                nc.sync.dma_start(out=ag_view(j0, nb), in_=sb[:])

            def mk_fin_bl1(b):
                def fin(sw, j0, nb, stage):
                    nc.sync.dma_start(
                        out=TBloc[b][:, j0 * D:(j0 + nb) * D],
                        in_=stage[:].rearrange("p b d -> p (b d)"))
                    sb = wp.tile([128, nb, D], bt, name="sb_b1", tag="sb16")
                    nc.vector.tensor_tensor(
                        out=sb[:], in0=stage[:],
                        in1=bcast(rs_b_t[b], j0, nb), op=OP.mult)
                    nc.sync.dma_start(out=ag_view(j0, nb), in_=sb[:])
                return fin

            def mk_fin_bl2(b):
                def fin(sw, j0, nb, stage):
                    tbl = wp.tile([128, nb, D], bt, name="tbl", tag="tbl")
                    nc.sync.dma_start(
                        out=tbl[:].rearrange("p b d -> p (b d)"),
                        in_=TBloc[b][:, j0 * D:(j0 + nb) * D])
                    st32 = wp.tile([128, nb, D], f32, name="st32b", tag="st32")
                    nc.vector.tensor_tensor(
                        out=st32[:], in0=stage[:], in1=acc[:, j0:j0 + nb, :], op=OP.add)
                    nc.vector.tensor_tensor(
                        out=st32[:], in0=st32[:], in1=tbl[:], op=OP.add)
                    stage = st32
                    nc.vector.tensor_scalar_mul(stage[:], stage[:], 1.0 / 3.0)
                    # spill user/item final rows (DVE cast + HWDGE dma; no
                    # SWDGE here -- Pool DMAs would desync the DMASW lane <->
                    # gather queue_num pairing)
                    sb16 = wp.tile([128, nb, D], bt, name="sb16", tag="sb16")
                    nc.vector.tensor_copy(out=sb16[:], in_=stage[:])
                    nc.sync.dma_start(
                        out=FBloc[b][:, j0 * D:(j0 + nb) * D],
                        in_=sb16[:].rearrange("p b d -> p (b d)"))
                    w = wp.tile([128, nb, D], bt, tag="fw")
                    nc.vector.tensor_tensor(
                        out=w[:], in0=stage[:], in1=bcast(wn_t[b], j0, nb), op=OP.mult)
                    nc.vector.tensor_tensor(
                        out=if_acc[:, j0:j0 + nb, :], in0=if_acc[:, j0:j0 + nb, :],
                        in1=w[:], op=OP.add)
                return fin

            # ---------------- passes
            if PH >= 1:
                emit_pass("g1", meta["lay_g"], False, t0g, rs_g_t, fin_g1)
                do_ag(T1)
            if PH >= 2:
                emit_pass("g2", meta["lay_g"], False, T1, rs_g_t, fin_g2)
                do_ag(AE)
            if PH >= 3:
                for b in range(B):
                    emit_pass(f"b{b}l1", meta["lay_b1"][b], True, AE,
                              rs_b_t[b], mk_fin_bl1(b))
                    do_ag(TB1[b])
            if PH >= 4:
                for b in range(B):
                    emit_pass(f"b{b}l2", meta["lay_b2"][b], False, TB1[b],
                              rs_b_t[b], mk_fin_bl2(b))
                if16 = wp.tile([128, NBLK, D], bt, name="if16", tag="if16")
                nc.vector.tensor_copy(out=if16[:], in_=if_acc[:])
                nc.sync.dma_start(
                    out=IFloc[:], in_=if16[:].rearrange("p b d -> p (b d)"))

            # ---------------- tail
            if PH >= 5:
                # stage build: users from FBloc[bb], items from IFloc
                ut = wp.tile([128, 3 * TP.u_grp // 128 * 8], i16, tag="ut")
                nc.vector.memset(ut[:], 0)
                nc.sync.dma_start(out=ut[0:16, :], in_=tu_idx[:])
                sg = wp.tile([128, TP.u_cols, 128], bt, tag="sg", bufs=1)
                gcol = TP.u_grp // 128
                for bb in range(B):
                    nc.gpsimd.dma_gather(
                        out_ap=sg[:, bb * gcol:(bb + 1) * gcol, :],
                        in_ap=FBloc[bb][:].rearrange(
                            "p (r c) -> (p r) c", c=128),
                        idxs_ap=ut[:, bb * TP.u_grp // 16:(bb + 1) * TP.u_grp // 16],
                        num_idxs=TP.u_grp, num_idxs_reg=TP.u_grp,
                        elem_size=128, single_packet=False, queue_num=next_q())
                upar_t = wp.tile([128, TP.u_cols], bt, tag="upar")
                nc.sync.dma_start(out=upar_t[:], in_=tu_par[:])
                ssel = wp.tile([128, TP.u_cols, D], bt, tag="ssel", bufs=1)
                dtmp = wp.tile([128, TP.u_cols, D], bt, tag="dtmp", bufs=1)
                nc.vector.tensor_tensor(
                    out=dtmp[:], in0=sg[:, :, D:2 * D], in1=sg[:, :, 0:D], op=OP.subtract)
                nc.vector.tensor_tensor(
                    out=dtmp[:], in0=dtmp[:],
                    in1=upar_t[:, :, None].to_broadcast([128, TP.u_cols, D]), op=OP.mult)
                nc.vector.tensor_tensor(
                    out=ssel[:], in0=sg[:, :, 0:D], in1=dtmp[:], op=OP.add)
                nc.sync.dma_start(
                    out=stage_u_in[:].rearrange("(p r) c -> p r c", p=128),
                    in_=ssel[:])
                nc.gpsimd.collective_compute(
                    "AllGather", OP.bypass, replica_groups=RG,
                    ins=[stage_u_in[:]], outs=[stage_u_full[:]])

                it = wp.tile([128, TP.i_grp // 128 * 8], i16, tag="it")
                nc.vector.memset(it[:], 0)
                nc.sync.dma_start(out=it[0:16, :], in_=ti_idx[:])
                sgi = wp.tile([128, TP.i_cols, 128], bt, tag="sgi", bufs=1)
                nc.gpsimd.dma_gather(
                    out_ap=sgi[:, :, :],
                    in_ap=IFloc[:].rearrange("p (r c) -> (p r) c", c=128),
                    idxs_ap=it[:, :TP.i_grp // 16],
                    num_idxs=TP.i_grp, num_idxs_reg=TP.i_grp,
                    elem_size=128, single_packet=False, queue_num=next_q())
                ipar_t = wp.tile([128, TP.i_cols], bt, tag="ipar")
                nc.sync.dma_start(out=ipar_t[:], in_=ti_par[:])
                isel = wp.tile([128, TP.i_cols, D], bt, tag="isel", bufs=1)
                ditmp = wp.tile([128, TP.i_cols, D], bt, tag="ditmp", bufs=1)
                nc.vector.tensor_tensor(
                    out=ditmp[:], in0=sgi[:, :, D:2 * D], in1=sgi[:, :, 0:D], op=OP.subtract)
                nc.vector.tensor_tensor(
                    out=ditmp[:], in0=ditmp[:],
                    in1=ipar_t[:, :, None].to_broadcast([128, TP.i_cols, D]), op=OP.mult)
                nc.vector.tensor_tensor(
                    out=isel[:], in0=sgi[:, :, 0:D], in1=ditmp[:], op=OP.add)
                nc.sync.dma_start(
                    out=stage_i_in[:].rearrange("(p r) c -> p r c", p=128),
                    in_=isel[:])
                nc.gpsimd.collective_compute(
                    "AllGather", OP.bypass, replica_groups=RG,
                    ins=[stage_i_in[:]], outs=[stage_i_full[:]])

            if PH >= 6:
                # slice gathers
                sut = wp.tile([128, TP.spc * 3 * 8], i16, tag="sut")
                nc.vector.memset(sut[:], 0)
                nc.sync.dma_start(out=sut[0:16, :], in_=su_idx[:])
                sgt = wp.tile([128, TP.spc * 3, 128], bt, tag="sgt", bufs=1)
                nc.gpsimd.dma_gather(
                    out_ap=sgt[:, :, :],
                    in_ap=stage_u_full[:].rearrange("(r two) c -> r (two c)", two=2),
                    idxs_ap=sut[:, :TP.spc * 3 * 8],
                    num_idxs=TP.spc * 3 * 128, num_idxs_reg=TP.spc * 3 * 128,
                    elem_size=128, single_packet=False, queue_num=next_q())
                supar_t = wp.tile([128, TP.spc * 3], bt, tag="supar")
                nc.sync.dma_start(out=supar_t[:], in_=su_par[:])
                su = wp.tile([128, TP.spc * 3, D], bt, tag="su", bufs=1)
                sd = wp.tile([128, TP.spc * 3, D], bt, tag="sd", bufs=1)
                nc.vector.tensor_tensor(
                    out=sd[:], in0=sgt[:, :, D:2 * D], in1=sgt[:, :, 0:D], op=OP.subtract)
                nc.vector.tensor_tensor(
                    out=sd[:], in0=sd[:],
                    in1=supar_t[:, :, None].to_broadcast([128, TP.spc * 3, D]), op=OP.mult)
                nc.vector.tensor_tensor(
                    out=su[:], in0=sgt[:, :, 0:D], in1=sd[:], op=OP.add)

                sit = wp.tile([128, TP.spc * 2 * 8], i16, tag="sit")
                nc.vector.memset(sit[:], 0)
                nc.sync.dma_start(out=sit[0:16, :], in_=si_idx[:])
                sgti = wp.tile([128, TP.spc * 2, 128], bt, tag="sgti", bufs=1)
                nc.gpsimd.dma_gather(
                    out_ap=sgti[:, :, :],
                    in_ap=stage_i_full[:].rearrange("(r two) c -> r (two c)", two=2),
                    idxs_ap=sit[:, :TP.spc * 2 * 8],
                    num_idxs=TP.spc * 2 * 128, num_idxs_reg=TP.spc * 2 * 128,
                    elem_size=128, single_packet=False, queue_num=next_q())
                sipar_t = wp.tile([128, TP.spc * 2], bt, tag="sipar")
                nc.sync.dma_start(out=sipar_t[:], in_=si_par[:])
                si = wp.tile([128, TP.spc * 2, D], bt, tag="si", bufs=1)
                sdi = wp.tile([128, TP.spc * 2, D], bt, tag="sdi", bufs=1)
                nc.vector.tensor_tensor(
                    out=sdi[:], in0=sgti[:, :, D:2 * D], in1=sgti[:, :, 0:D], op=OP.subtract)
                nc.vector.tensor_tensor(
                    out=sdi[:], in0=sdi[:],
                    in1=sipar_t[:, :, None].to_broadcast([128, TP.spc * 2, D]), op=OP.mult)
                nc.vector.tensor_tensor(
                    out=si[:], in0=sgti[:, :, 0:D], in1=sdi[:], op=OP.add)

                loss_acc = mp.tile([128, 1], f32)
                nc.vector.memset(loss_acc[:], 0.0)
                slw_t = wp.tile([128, TP.spc * 3], f32, tag="slw")
                nc.sync.dma_start(out=slw_t[:], in_=slw_p[:])
                # S column index for (i,j) i<=j: (0,0)(0,1)(0,2)(1,1)(1,2)(2,2)
                SIX = {(0, 0): 0, (0, 1): 1, (0, 2): 2, (1, 1): 3,
                       (1, 2): 4, (2, 2): 5}
                for sr in range(TP.spc):
                    fu = [su[:, sr * 3 + bb, :] for bb in range(3)]
                    itf = [si[:, sr * 2 + k, :] for k in range(2)]
                    S6 = wp.tile([128, 6], f32, tag="S6")
                    scr = wp.tile([128, D], f32, tag="scr")
                    for (i, j), col in SIX.items():
                        nc.vector.tensor_tensor(
                            out=scr[:], in0=fu[i], in1=fu[j], op=OP.mult)
                        nc.vector.tensor_reduce(
                            out=S6[:, col:col + 1], in_=scr[:],
                            axis=mybir.AxisListType.X, op=OP.add)
                    lastc = [2, 4, 5]
                    fj = wp.tile([128, 3], f32, tag="fj")
                    sq = wp.tile([128, 3], f32, tag="sq")
                    den = wp.tile([128, 3], f32, tag="den")
                    for j in range(3):
                        nc.vector.tensor_tensor(
                            out=sq[:, j:j + 1], in0=S6[:, lastc[j]:lastc[j] + 1],
                            in1=S6[:, lastc[j]:lastc[j] + 1], op=OP.mult)
                    nc.vector.tensor_scalar_add(den[:], sq[:], 1e-12)
                    nc.vector.reciprocal(out=den[:], in_=den[:])
                    nc.vector.tensor_tensor(out=fj[:], in0=sq[:], in1=den[:], op=OP.mult)
                    # clear[i][j] = S[min,max]*fj[j]; rowv_j = w0*cl0j+w1*cl1j+w2*last_j
                    rowv = wp.tile([128, 3], f32, tag="rowv")
                    tmpc = wp.tile([128, 3], f32, tag="tmpc")
                    cl0c = [0, 1, 2]
                    cl1c = [1, 3, 4]
                    for j in range(3):
                        nc.vector.tensor_tensor(
                            out=rowv[:, j:j + 1], in0=S6[:, cl0c[j]:cl0c[j] + 1],
                            in1=fj[:, j:j + 1], op=OP.mult)
                        nc.vector.tensor_tensor(
                            out=tmpc[:, j:j + 1], in0=S6[:, cl1c[j]:cl1c[j] + 1],
                            in1=fj[:, j:j + 1], op=OP.mult)
                    w0 = slw_t[:, sr * 3 + 0:sr * 3 + 1]
                    w1 = slw_t[:, sr * 3 + 1:sr * 3 + 2]
                    w2 = slw_t[:, sr * 3 + 2:sr * 3 + 3]
                    nc.vector.tensor_scalar(
                        out=rowv[:], in0=rowv[:], scalar1=w0, scalar2=None, op0=OP.mult)
                    nc.vector.tensor_scalar(
                        out=tmpc[:], in0=tmpc[:], scalar1=w1, scalar2=None, op0=OP.mult)
                    nc.vector.tensor_tensor(out=rowv[:], in0=rowv[:], in1=tmpc[:], op=OP.add)
                    lastv = wp.tile([128, 3], f32, tag="lastv")
                    for j in range(3):
                        nc.vector.tensor_copy(
                            out=lastv[:, j:j + 1], in_=S6[:, lastc[j]:lastc[j] + 1])
                    nc.vector.tensor_scalar(
                        out=lastv[:], in0=lastv[:], scalar1=w2, scalar2=None, op0=OP.mult)
                    nc.vector.tensor_tensor(out=rowv[:], in0=rowv[:], in1=lastv[:], op=OP.add)
                    ev = wp.tile([128, 3], f32, tag="ev")
                    nc.scalar.activation(out=ev[:], in_=rowv[:], func=AF.Exp, scale=0.125)
                    esum = wp.tile([128, 1], f32, tag="esum")
                    nc.vector.tensor_reduce(
                        out=esum[:], in_=ev[:], axis=mybir.AxisListType.X, op=OP.add)
                    nc.vector.reciprocal(out=esum[:], in_=esum[:])
                    att = wp.tile([128, 3], f32, tag="att")
                    nc.vector.tensor_scalar(
                        out=att[:], in0=ev[:], scalar1=esum[:], scalar2=None, op0=OP.mult)
                    uf = wp.tile([128, D], f32, tag="uf")
                    uft = wp.tile([128, D], f32, tag="uft")
                    nc.vector.tensor_scalar(
                        out=uf[:], in0=fu[0], scalar1=att[:, 0:1], scalar2=None, op0=OP.mult)
                    for j in (1, 2):
                        nc.vector.tensor_scalar(
                            out=uft[:], in0=fu[j], scalar1=att[:, j:j + 1],
                            scalar2=None, op0=OP.mult)
                        nc.vector.tensor_tensor(out=uf[:], in0=uf[:], in1=uft[:], op=OP.add)
                    sc = wp.tile([128, 2], f32, tag="sc")
                    for k in range(2):
                        nc.vector.tensor_tensor(
                            out=scr[:], in0=uf[:], in1=itf[k], op=OP.mult)
                        nc.vector.tensor_reduce(
                            out=sc[:, k:k + 1], in_=scr[:],
                            axis=mybir.AxisListType.X, op=OP.add)
                    dd = wp.tile([128, 1], f32, tag="dd")
                    nc.vector.tensor_tensor(
                        out=dd[:], in0=sc[:, 0:1], in1=sc[:, 1:2], op=OP.subtract)
                    sg = wp.tile([128, 1], f32, tag="sg", bufs=1)
                    nc.scalar.activation(out=sg[:], in_=dd[:], func=AF.Sigmoid)
                    nc.vector.tensor_scalar_add(sg[:], sg[:], 1e-10)
                    lg = wp.tile([128, 1], f32, tag="lg")
                    nc.scalar.activation(out=lg[:], in_=sg[:], func=AF.Ln)
                    nc.vector.tensor_tensor(
                        out=loss_acc[:], in0=loss_acc[:], in1=lg[:], op=OP.add)

                ps1 = pp.tile([1, 1], f32, tag="ps1")
                nc.tensor.matmul(out=ps1[:], lhsT=ones_f[:], rhs=loss_acc[:],
                                 start=True, stop=True)
                red = mp.tile([1, 1], f32)
                nc.vector.tensor_copy(out=red[:], in_=ps1[:])
                nc.vector.tensor_scalar_mul(red[:], red[:], -1.0 / C.BATCH)
                nc.sync.dma_start(out=loss_in[:], in_=red[:])
                nc.gpsimd.collective_compute(
                    "AllReduce", OP.add, replica_groups=RG,
                    ins=[loss_in[:]], outs=[loss_out[:]])
                fin_t = mp.tile([1, 1], f32)
                nc.sync.dma_start(out=fin_t[:], in_=loss_out[:])
                nc.sync.dma_start(out=out_p[:], in_=fin_t[:])
            else:
                zz = mp.tile([1, 1], f32)
                nc.vector.memset(zz[:], 0.0)
                nc.sync.dma_start(out=out_p[:], in_=zz[:])

    nc.finalize()
    return nc


# ================================================================ entry

def _install_loud_hook():
    try:
        import traceback
        from concourse import bass2jax
        import libneuronxla
        orig = bass2jax.neuronx_cc_hook

        def loud(*a, **k):
            try:
                return orig(*a, **k)
            except BaseException:
                traceback.print_exc()
                raise
        if not hasattr(libneuronxla, "orig_neuronx_cc"):
            libneuronxla.orig_neuronx_cc = libneuronxla.neuronx_cc
        libneuronxla.neuronx_cc = loud
        bass2jax.neuronx_cc_hook = loud
    except Exception:
        pass


def _in_maps(meta):
    C = meta["cfg"]
    TP = meta["tail"]
    maps = []
    for c in range(C.NCORES):
        m = {
            "t0g": meta["T0g"],
            "t0acc": meta["t0acc"][c],
            "idx_g1": meta["lay_g"].idx[c],
            "dl_g1": meta["lay_g"].dl[c],
            "rs_g_blk": meta["rs_g_blk"][c],
            "tu_idx": TP.u_idx[c],
            "tu_par": TP.u_par[c],
            "ti_idx": TP.i_idx[c],
            "ti_par": TP.i_par[c],
            "su_idx": TP.slice_u_idx[c],
            "su_par": TP.slice_u_par[c],
            "si_idx": TP.slice_i_idx[c],
            "si_par": TP.slice_i_par[c],
            "slw": TP.slw[c],
        }
        for b in range(C.B):
            m[f"idx_b{b}l1"] = meta["lay_b1"][b].idx[c]
            m[f"dl_b{b}l1"] = meta["lay_b1"][b].dl[c]
            m[f"rs_b{b}l1"] = meta["lay_b1"][b].rse[c]
            m[f"idx_b{b}l2"] = meta["lay_b2"][b].idx[c]
            m[f"dl_b{b}l2"] = meta["lay_b2"][b].dl[c]
            m[f"rs_b{b}_blk"] = meta["rs_b_blk"][b][c]
            m[f"wn{b}_blk"] = meta["wn_blk"][b][c]
        maps.append(m)
    return maps


def kernel(**inputs):
    import concourse.tile  # noqa: F401
    from concourse.bass_utils import run_bass_kernel_spmd
    _install_loud_hook()
    cfg = FULL
    meta = _host_prep(cfg, inputs["user_emb"], inputs["item_emb"], inputs["W"],
                      inputs["edge_users"], inputs["edge_items"],
                      inputs["batch_data"])
    nc = _build_graph(meta)
    maps = _in_maps(meta)
    if os.environ.get("GNN_BUILD_ONLY"):
        print("BUILD ONLY: n_inst =", len(nc.inst_map))
        return np.float32(0.0)
    res = run_bass_kernel_spmd(
        nc, maps, core_ids=list(range(cfg.NCORES)),
        trace=bool(os.environ.get("GNN_TRACE")))
    kernel.last_result = res
    dev = float(res.results[0]["out"][0, 0])
    return np.float32(dev + meta["reg"])

